# revision 51
# baseline (speedup 1.0000x reference)
"""Trainium2 Bass kernel for ActorMoE (8 experts, dims 512->1024->512->256->64).

Strategy: data-parallel across 8 NeuronCores (2048 rows each), weights
replicated. On-device compute is feature-major (features on partitions,
batch on the free dim) so the stacked expert weights W_l[e] (shape
[in, out]) are directly the matmul lhsT and no transposes are needed.

fp8: layers 0-2 run as fp8e4m3 DoubleRow matmuls (PE streams 2 k-rows per
cycle, ~2.1x bf16 throughput measured). Weights are host-scaled by 64 so
their uniform(+-1/sqrt(fan_in)) range uses fp8 normals; the 1/64 descale
folds into the ELU's activation scale. Hidden activations h1/h2 are stored
fp8 (quantizing elu directly, which is exact near -1 and relatively
accurate elsewhere); h3 stays bf16 for the bf16 L3. End-to-end rel err vs
f32 reference simulates to 1.3e-2 (budget 2e-2).

ELU trick (one ACT + one DVE pass per element, no table-ELU on TRN2):
  e = Exp(z*s + b)                  (ScalarE, scale+bias fused)
  h = min(e - 1, relu(z*s + b))     (custom DVE op ELU_SC_MOE)
exact because e^u - 1 >= u for u >= 0.

L3 + gate weighting: per expert, DVE computes (z + b3) * w_e from PSUM in
one scalar_tensor_tensor, and the running sum over experts is an f32
tensor_tensor add on the otherwise-idle Pool/GPSIMD engine. Keeping the
output accumulator in SBUF (not PSUM) leaves all 8 PSUM banks for the
matmul-group rotation (4 groups in flight), which removes layer-boundary
PE stalls waiting on ELU drains — worth ~15us over the 2-bank-resident
variant.

Matmuls are emitted with same-weight run length 4 (both 2-bank PSUM groups
of an m-tile accumulate together). Expert layers are software-pipelined
(tick e: load(e+1), L0(e), L2(e-1), L1(e), L3(e-1)) so each layer-boundary
ELU drain is covered by other matmul work; the gate fills the first
boundary.

Softmax gate: logits are small (|logit| < ~2) so exp without max-shift is
safe. wT rows are replicated across partitions via broadcast DMA (bounced
through DRAM, since partition-broadcast needs a DRAM source).
"""

import sys

sys.path.insert(0, "/opt/trn_rl_repo")

import numpy as np
import ml_dtypes

BF = ml_dtypes.bfloat16
E4 = ml_dtypes.float8_e4m3

B, OBS, ACT, E = 16384, 512, 64, 8
DIMS = [512, 1024, 512, 256, 64]
GH = 256
NCORES = 8
BSH = B // NCORES  # 2048
P = 128
FD = 512  # matmul free dim (one PSUM bank of f32)
NT = BSH // FD  # 4 n-tiles per core
NB = 2  # PSUM banks per group (ELU op width = NB*FD)
NG = NT // NB  # groups per m-tile
WS = 64.0  # fp8 weight scale

_cache = {}
_PIPE = {"on": True}  # software-pipelined expert/layer emission order
_STRIP = {}  # timing ablations only: {"elu": True} or {"act": True}


def _get_elu_op():
    """Custom DVE op: out = min(in1 - 1, max(in0*s1 + s0, 0)).
    With in0 = z (PSUM), s1 = 1/WS, s0 = b per-partition, in1 = exp(z*s1+b)
    from ACT, this computes elu(z*s1+b) in a single DVE pass."""
    if "elu_op" in _cache:
        return _cache["elu_op"]
    from concourse.dve_ops import DveOp, OPS
    from concourse.dve_spec import (
        Spec, Src0, Src1, C0, C1, Zero, One, maxx, minn, lower,
    )
    from concourse.dve_uop import DveOpSpec

    spec = Spec(
        body=minn(Src1 - One, maxx(Src0 * C1 + C0, Zero)),
        reference=lambda in0, in1, s0, s1: np.minimum(
            in1 - 1.0, np.maximum(in0 * s1 + s0, 0.0)
        ),
    )
    shas = {}
    for ver in ("v3", "v4"):
        s = DveOpSpec(name="ELU_SC_MOE", opcode=0, uops=lower(spec, ver=ver), rd1_en=True)
        shas[ver] = s.sha(ver)
    op = DveOp("ELU_SC_MOE", spec, subdim=False, uops_sha=shas)
    OPS.append(op)
    # import-time lookup tables don't see post-import appends — patch them
    import concourse.dve_ops as dve_ops_mod

    dve_ops_mod.CUSTOM_DVE_SPECS[op.name] = op.spec
    dve_ops_mod._SUB_OPCODE_FOR_NAME[op.name] = (
        dve_ops_mod._CUSTOM_DVE_ROW_BASE + len(OPS) - 1
    )
    _cache["elu_op"] = op
    return op


def _build(reps=1):
    """Build the Bass graph. reps>1 wraps the whole body in a For_i loop
    (the body is idempotent) — used only for timing via wall-time slope."""
    import concourse.bass as bass  # noqa: F401
    from concourse import bacc, mybir
    import concourse.tile as tile

    f32 = mybir.dt.float32
    bf16 = mybir.dt.bfloat16
    fp8 = mybir.dt.float8e4
    AF = mybir.ActivationFunctionType
    Alu = mybir.AluOpType
    DR = mybir.MatmulPerfMode.DoubleRow

    nc = bacc.Bacc(None, target_bir_lowering=False)

    xTd = nc.dram_tensor("xT", [OBS, BSH], bf16, kind="ExternalInput")
    xT8d = nc.dram_tensor("xT8", [OBS, BSH], fp8, kind="ExternalInput")
    Wd = [
        nc.dram_tensor(
            f"W{l}", [E, DIMS[l], DIMS[l + 1]], fp8 if l < 3 else bf16,
            kind="ExternalInput",
        )
        for l in range(4)
    ]
    # packed biases: [E, 128, MT] with b[e, p, mo] = bias[e, mo*128 + p]
    MTS = [DIMS[l + 1] // P for l in range(3)]  # [8, 4, 2]
    Bd = [
        nc.dram_tensor(f"B{l}", [E, P, MTS[l]], f32, kind="ExternalInput")
        for l in range(3)
    ]
    B3d = nc.dram_tensor("B3", [E, ACT, 1], f32, kind="ExternalInput")
    gW0d = nc.dram_tensor("gW0", [OBS, GH], bf16, kind="ExternalInput")
    gW1d = nc.dram_tensor("gW1", [GH, E], bf16, kind="ExternalInput")
    gB0d = nc.dram_tensor("gB0", [P, GH // P], f32, kind="ExternalInput")
    gB1d = nc.dram_tensor("gB1", [E, 1], f32, kind="ExternalInput")
    outd = nc.dram_tensor("out", [ACT, BSH], f32, kind="ExternalOutput")

    with tile.TileContext(nc) as tc:
        with (
            tc.tile_pool(name="const", bufs=1) as cpool,
            tc.tile_pool(name="xpool", bufs=2) as xpool,
            tc.tile_pool(name="wpool", bufs=2) as wpool,
            tc.tile_pool(name="wpool3", bufs=3) as wpool3,
            tc.tile_pool(name="bpool", bufs=3) as bpool,
            tc.tile_pool(name="hpool", bufs=1) as hpool,
            tc.tile_pool(name="epool", bufs=8) as epool,
            tc.tile_pool(name="tpool", bufs=2) as tpool,
            tc.tile_pool(name="psum", bufs=4, space="PSUM") as pspool,
            tc.tile_pool(name="dram", bufs=2, space="DRAM") as dpool,
        ):

            def body():
                # ---- load x and gate params ----
                gw0 = cpool.tile([P, OBS // P, GH], bf16, tag="gw0", name="gw0")
                nc.sync.dma_start(gw0[:], gW0d[:].rearrange("(ko p) o -> p ko o", p=P))
                xt = xpool.tile([P, OBS // P, BSH], bf16, tag="xt", name="xt")
                xt_src = xTd[:].rearrange("(ko p) n -> p ko n", p=P)
                for ko in range(OBS // P):
                    # split across two DGE queues so the gate isn't DMA-bound
                    eng = nc.sync if ko % 2 == 0 else nc.scalar
                    eng.dma_start(xt[:, ko : ko + 1, :], xt_src[:, ko : ko + 1, :])
                xt8 = xpool.tile([P, OBS // P, BSH], fp8, tag="xt8", name="xt8")
                xt8_src = xT8d[:].rearrange("(ko p) n -> p ko n", p=P)
                for ko in range(OBS // P):
                    eng = nc.sync if ko % 2 == 0 else nc.scalar
                    eng.dma_start(xt8[:, ko : ko + 1, :], xt8_src[:, ko : ko + 1, :])
                gw1 = cpool.tile([P, GH // P, E], bf16, tag="gw1", name="gw1")
                nc.sync.dma_start(gw1[:], gW1d[:].rearrange("(ko p) o -> p ko o", p=P))
                gb0t = cpool.tile([P, GH // P], f32, tag="gb0", name="gb0")
                nc.scalar.dma_start(gb0t[:], gB0d[:])
                gb1t = cpool.tile([E, 1], f32, tag="gb1", name="gb1")
                nc.scalar.dma_start(gb1t[:], gB1d[:])


                elu_op = _get_elu_op()

                def elu_wide(ps_flat, bias_ap, out_ap, scale, mp=P):
                    # ps_flat: [mp, NB*FD] PSUM view; one wide ACT + one wide DVE
                    et = epool.tile([P, NB * FD], bf16, tag="e", name="e")[:mp]
                    if _STRIP.get("act"):
                        nc.gpsimd.memset(et[:, 0:8], 0.5)
                    else:
                        nc.scalar.activation(
                            et, ps_flat, AF.Exp, bias=bias_ap, scale=scale
                        )
                    if _STRIP.get("elu"):
                        nc.gpsimd.memset(out_ap[:, 0:8], 0.25)
                        return
                    # fused custom DVE: out = min(et-1, relu(z*scale + b)) = elu
                    nc.vector._custom_dve(
                        elu_op, out=out_ap, in0=ps_flat, in1=et, s0=bias_ap, s1=scale
                    )

                def psum_mm_groups(win_col, rhs_tile, KT, mp=P):
                    """bf16 path: all NG groups of one m-tile accumulated
                    together so each weight load serves NT consecutive matmuls
                    (same-weight run length 4)."""
                    psts = [
                        pspool.tile([P, NB, FD], f32, tag="ps", name="ps")
                        for _ in range(NG)
                    ]
                    for k in range(KT):
                        lhs = win_col(k)
                        for g in range(NG):
                            for n in range(NB):
                                ng = g * NB + n
                                nc.tensor.matmul(
                                    psts[g][:mp, n, :],
                                    lhs,
                                    rhs_tile[:, k, ng * FD : (ng + 1) * FD],
                                    start=(k == 0),
                                    stop=(k == KT - 1),
                                )
                    return [pst[:mp].rearrange("p a b -> p (a b)") for pst in psts]

                def psum_mm_groups_dr(win, m, rhs_tile, KP):
                    """fp8 DoubleRow path: each instruction contracts a k-pair
                    (K=256) with the full 128-col stationary array."""
                    psts = [
                        pspool.tile([P, NB, FD], f32, tag="ps", name="ps")
                        for _ in range(NG)
                    ]
                    for kp in range(KP):
                        lhs = win[:, 2 * kp : 2 * kp + 2, m * P : (m + 1) * P]
                        for g in range(NG):
                            for n in range(NB):
                                ng = g * NB + n
                                nc.tensor.matmul(
                                    psts[g][:, n, :],
                                    lhs,
                                    rhs_tile[:, 2 * kp : 2 * kp + 2,
                                             ng * FD : (ng + 1) * FD],
                                    start=(kp == 0),
                                    stop=(kp == KP - 1),
                                    perf_mode=DR,
                                )
                    return [pst.rearrange("p a b -> p (a b)") for pst in psts]

                def layer8(win, bt, KP, MT, rhs_tile, out_tile):
                    """fp8 layer: z = win.T @ rhs (DoubleRow); out = elu (fp8/bf16)."""
                    for m in range(MT):
                        flats = psum_mm_groups_dr(win, m, rhs_tile, KP)
                        for g in range(NG):
                            elu_wide(
                                flats[g],
                                bt[:, m : m + 1],
                                out_tile[:, m, g * NB * FD : (g + 1) * NB * FD],
                                1.0 / WS,
                            )

                def emit_gate_l1():
                    # gate layer 1 (512 -> 256, elu), bf16
                    gp = cpool.tile([P, GH // P, BSH], bf16, tag="gp", name="gp")
                    for m in range(GH // P):
                        flats = psum_mm_groups(
                            lambda k, m=m: gw0[:, k, m * P : (m + 1) * P], xt, OBS // P
                        )
                        for g in range(NG):
                            elu_wide(
                                flats[g],
                                gb0t[:, m : m + 1],
                                gp[:, m, g * NB * FD : (g + 1) * NB * FD],
                                1.0,
                            )
                    return gp

                def emit_gate_rest(gp):
                    # gate layer 2 (256 -> 8) + exp
                    expT = cpool.tile([E, BSH], f32, tag="expT", name="expT")
                    gflats = psum_mm_groups(lambda k: gw1[:, k, :], gp, GH // P, mp=E)
                    for g in range(NG):
                        nc.scalar.activation(
                            expT[:, g * NB * FD : (g + 1) * NB * FD],
                            gflats[g],
                            AF.Exp,
                            bias=gb1t[:, 0:1],
                        )
                    # softmax denom: sum over 8 experts via ones-matmul
                    ones = cpool.tile([E, 1], f32, tag="ones", name="ones")
                    nc.vector.memset(ones[:], 1.0)
                    invs = cpool.tile([1, BSH], f32, tag="invs", name="invs")
                    sflats = psum_mm_groups(lambda k: ones[:], expT[:, None, :], 1, mp=1)
                    for g in range(NG):
                        nc.vector.reciprocal(
                            invs[:, g * NB * FD : (g + 1) * NB * FD], sflats[g]
                        )
                    # wT[e, s] = exp(logit_e)/sum (partition-broadcast DMA
                    # needs a DRAM source, so bounce via DRAM)
                    inv_d = dpool.tile([1, BSH], f32, name="inv_d")
                    nc.scalar.dma_start(inv_d[:], invs[:])
                    rep8 = cpool.tile([E, BSH], f32, tag="rep8", name="rep8")
                    nc.scalar.dma_start(
                        rep8[:], inv_d[0:1, :].to_broadcast((E, BSH))
                    )
                    wT = cpool.tile([E, BSH], f32, tag="wT", name="wT")
                    nc.vector.tensor_tensor(wT[:], expT[:], rep8[:], Alu.mult)
                    wt_d = dpool.tile([E, BSH], f32, name="wt_d")
                    nc.scalar.dma_start(wt_d[:], wT[:])
                    return wT, wt_d

                def load_expert(e):
                    st = {}
                    st["w0"] = wpool.tile(
                        [P, DIMS[0] // P, DIMS[1]], fp8, tag="w0", name="w0"
                    )
                    nc.sync.dma_start(
                        st["w0"][:], Wd[0][e].rearrange("(ko p) o -> p ko o", p=P)
                    )
                    st["w1"] = wpool.tile(
                        [P, DIMS[1] // P, DIMS[2]], fp8, tag="w1", name="w1"
                    )
                    nc.sync.dma_start(
                        st["w1"][:], Wd[1][e].rearrange("(ko p) o -> p ko o", p=P)
                    )
                    st["w2"] = wpool3.tile(
                        [P, DIMS[2] // P, DIMS[3]], fp8, tag="w2", name="w2"
                    )
                    nc.sync.dma_start(
                        st["w2"][:], Wd[2][e].rearrange("(ko p) o -> p ko o", p=P)
                    )
                    st["w3"] = wpool3.tile(
                        [P, DIMS[3] // P, DIMS[4]], bf16, tag="w3", name="w3"
                    )
                    nc.sync.dma_start(
                        st["w3"][:], Wd[3][e].rearrange("(ko p) o -> p ko o", p=P)
                    )
                    bts = []
                    for l in range(3):
                        bt = bpool.tile([P, MTS[l]], f32, tag=f"b{l}", name=f"b{l}")
                        nc.scalar.dma_start(bt[:], Bd[l][e])
                        bts.append(bt)
                    st["bts"] = bts
                    st["b3"] = bpool.tile([ACT, 1], f32, tag="b3", name="b3")
                    nc.scalar.dma_start(st["b3"][:], B3d[e])
                    return st

                def emit_rw(st, e, wt_d):
                    # gate row for this expert, replicated over 64 partitions
                    st["rw"] = wpool.tile([ACT, BSH], f32, tag="rw", name="rw")
                    nc.scalar.dma_start(
                        st["rw"][:], wt_d[e : e + 1, :].to_broadcast((ACT, BSH))
                    )

                def emit_L0(st):
                    st["h1"] = hpool.tile(
                        [P, DIMS[1] // P, BSH], fp8, tag="h1", name="h1"
                    )
                    layer8(st["w0"], st["bts"][0], DIMS[0] // (2 * P),
                           DIMS[1] // P, xt8, st["h1"])

                def emit_L1(st):
                    st["h2"] = hpool.tile(
                        [P, DIMS[2] // P, BSH], fp8, tag="h2", name="h2"
                    )
                    layer8(st["w1"], st["bts"][1], DIMS[1] // (2 * P),
                           DIMS[2] // P, st["h1"], st["h2"])

                def emit_L2(st):
                    st["h3"] = hpool.tile(
                        [P, DIMS[3] // P, BSH], bf16, tag="h3", name="h3"
                    )
                    layer8(st["w2"], st["bts"][2], DIMS[2] // (2 * P),
                           DIMS[3] // P, st["h2"], st["h3"])

                def emit_L3(st, e, acc):
                    # last layer (256 -> 64), no ELU. The weighted accumulation
                    # acc += (z + b3) * w_e runs on the otherwise-idle Pool
                    # engine (f32 only — the documented/measured gpsimd path),
                    # keeping DVE free and all 8 psum banks in rotation.
                    l3flats = psum_mm_groups(
                        lambda k: st["w3"][:, k, :], st["h3"], DIMS[3] // P, mp=ACT
                    )
                    for g in range(NG):
                        gs = slice(g * NB * FD, (g + 1) * NB * FD)
                        if e == 0:
                            nc.vector.scalar_tensor_tensor(
                                acc[:, gs], l3flats[g], st["b3"][:, 0:1],
                                st["rw"][:, gs], Alu.add, Alu.mult,
                            )
                        else:
                            tt = tpool.tile([ACT, NB * FD], f32, tag="t", name="t")
                            nc.vector.scalar_tensor_tensor(
                                tt, l3flats[g], st["b3"][:, 0:1],
                                st["rw"][:, gs], Alu.add, Alu.mult,
                            )
                            # f32 add on the otherwise-idle Pool engine (the
                            # documented/measured gpsimd tensor_tensor path)
                            nc.gpsimd.tensor_tensor(
                                acc[:, gs], acc[:, gs], tt, Alu.add
                            )

                acc = cpool.tile([ACT, BSH], f32, tag="acc", name="acc")

                if not _PIPE.get("on", True):
                    # serial reference order
                    wT, wt_d = emit_gate_rest(emit_gate_l1())
                    for e in range(E):
                        st = load_expert(e)
                        emit_rw(st, e, wt_d)
                        emit_L0(st)
                        emit_L1(st)
                        emit_L2(st)
                        emit_L3(st, e, acc)
                else:
                    # software pipeline: each layer-boundary ELU drain is
                    # covered by another expert's (or the gate's) matmuls.
                    # tick e: load(e+1), L0(e), [gate|L2(e-1)], rw(e), L1(e),
                    #         L3(e-1)
                    state = {0: load_expert(0)}
                    wt_d = None
                    for e in range(E + 1):
                        if e == 0:
                            gp = emit_gate_l1()
                        if e < E:
                            if e + 1 < E:
                                state[e + 1] = load_expert(e + 1)
                            emit_L0(state[e])
                        if e == 0:
                            wT, wt_d = emit_gate_rest(gp)
                        if e >= 1:
                            emit_L2(state[e - 1])
                        if e < E:
                            emit_rw(state[e], e, wt_d)
                            emit_L1(state[e])
                        if e >= 1:
                            emit_L3(state[e - 1], e - 1, acc)
                            del state[e - 1]

                nc.scalar.dma_start(outd[:], acc[:])

            if reps == 1:
                body()
            else:
                with tc.For_i(0, reps, 1):
                    body()

    nc.compile()
    return nc


def _prep_inputs(inputs):
    """Host-side: shard/transposes/casts. Returns in_maps."""
    x = np.asarray(inputs["x"], np.float32)
    Ws = [np.asarray(inputs[f"W{l}"], np.float32) for l in range(4)]
    bs = [np.asarray(inputs[f"b{l}"], np.float32) for l in range(4)]
    gW0 = np.asarray(inputs["gW0"], np.float32)
    gb0 = np.asarray(inputs["gb0"], np.float32)
    gW1 = np.asarray(inputs["gW1"], np.float32)
    gb1 = np.asarray(inputs["gb1"], np.float32)

    shared = {}
    for l in range(3):
        shared[f"W{l}"] = np.ascontiguousarray((Ws[l] * WS).astype(E4))
    shared["W3"] = np.ascontiguousarray(Ws[3].astype(BF))
    MTS = [DIMS[l + 1] // P for l in range(3)]
    for l in range(3):
        pk = bs[l].reshape(E, MTS[l], P).transpose(0, 2, 1)
        shared[f"B{l}"] = np.ascontiguousarray(pk)
    shared["B3"] = np.ascontiguousarray(bs[3][:, :, None])
    shared["gW0"] = np.ascontiguousarray(gW0.astype(BF))
    shared["gW1"] = np.ascontiguousarray(gW1.astype(BF))
    shared["gB0"] = np.ascontiguousarray(gb0.reshape(GH // P, P).T)
    shared["gB1"] = np.ascontiguousarray(gb1[:, None])

    in_maps = []
    for c in range(NCORES):
        m = dict(shared)
        xs = x[c * BSH : (c + 1) * BSH].T
        m["xT"] = np.ascontiguousarray(xs.astype(BF))
        m["xT8"] = np.ascontiguousarray(xs.astype(E4))
        in_maps.append(m)
    return in_maps


def kernel(**inputs):
    from concourse.bass_utils import run_bass_kernel_spmd

    if "nc" not in _cache:
        _cache["nc"] = _build()
    nc = _cache["nc"]
    in_maps = _prep_inputs(inputs)
    res = run_bass_kernel_spmd(nc, in_maps, core_ids=list(range(NCORES)))
    full = np.empty((B, ACT), np.float32)
    for c in range(NCORES):
        full[c * BSH : (c + 1) * BSH] = np.asarray(res.results[c]["out"]).T
    return full
